# Initial kernel scaffold
#
"""EquivariantBlock (EGNN) Trainium2 kernel — 8-core SPMD.

Sharding: nodes by destination row (6250/core, padded to 6272). Each core owns
the edges whose aggregation target (row) lands in its slice. CPU-side prep
sorts each core's edges by (col-half, row-window) and pads per-(half,window)
sections to shared quotas so one compiled program serves all 8 cores.

Device pipeline per core:
  prep : gather x[row],x[col] (fp32, edge-major) -> radial, coord_diff,
         radial^T -> ea table in HBM
  GCL x2: gather h[row],h[col] (bf16, transposed -> feature-major), edge MLP
         on TensorE, attention via K-sliced matmuls, ef -> edge-major via PE
         transpose, segment-sum via selection-matrix matmuls into per-window
         PSUM accumulated into agg^T (feature-major) in SBUF, node MLP,
         h residual update, rebuild bf16 h-table, AllGather across cores
  coord: same edge MLP with coord weights, phi * coord_diff * mask
         aggregated the same way, x update.
"""

import numpy as np
import ml_dtypes

import concourse.bass as bass
import concourse.mybir as mybir
from concourse.tile import TileContext
from concourse.bass_utils import run_bass_kernel_spmd
from concourse.masks import make_identity
from concourse import library_config
from concourse.library_overlay import lower_extended_insts

BF16 = mybir.dt.bfloat16
F32 = mybir.dt.float32
AF = mybir.ActivationFunctionType

N = 50000
E = 800000
H = 128
NCORES = 8
NSR = 6250          # real rows per core
NSP = 6272          # padded rows per core (49 * 128)
NWIN = NSP // 128   # 49 windows
TPAD = NCORES * NSP  # 50176 padded global table rows
HALF = 32768        # int16-safe table split
EPS = 1e-8
NORM_FACTOR = 100.0
GCHUNK = 2048       # idxs per dma_gather
ETILE = 512         # edges per MLP tile


def _wait_split_counter():
    c = [0]
    def fresh():
        c[0] += 1
        return f"WSPLIT-{c[0]}"
    return fresh


def split_excess_waits(nc, max_keep=1):
    """This container's walrus rejects instructions with >2 sem waits
    ("Too many sync wait commands"). Hoist extras onto same-engine nops."""
    fresh = _wait_split_counter()
    total = 0
    for fn in nc.m.functions:
        for bb in fn.blocks:
            out, changed = [], False
            for inst in bb.instructions:
                si = inst.sync_info
                if si is not None and si.on_wait and len(si.on_wait) > max_keep \
                        and inst.engine != mybir.EngineType.Unassigned:
                    hoist = [w for w in si.on_wait
                             if w.sync_type == "semaphore" and w.wait_reg is None]
                    keep = [w for w in si.on_wait
                            if not (w.sync_type == "semaphore" and w.wait_reg is None)]
                    while len(hoist) + len(keep) > max_keep and hoist:
                        w = hoist.pop(0)
                        nop = mybir.InstNoOp(name=fresh(), ins=[], outs=[])
                        nop.engine = inst.engine
                        nop.sync_info = mybir.SyncInfo(on_wait=[w], on_update=[])
                        out.append(nop)
                        total += 1
                        changed = True
                    si.on_wait = hoist + keep
                out.append(inst)
            if changed:
                bb.instructions = out
    return total


def pack_idx16(idx, epad):
    """int16 idx tile [128, epad/16]; position i -> (i%16, i//16), replicated
    across the 8 groups of 16 partitions."""
    buf = np.zeros(epad, np.int16)
    buf[:idx.shape[0]] = idx.astype(np.int16)
    t = buf.reshape(-1, 16).T
    return np.ascontiguousarray(np.tile(t, (8, 1)))


# ---------------------------------------------------------------- CPU shard

def shard(inputs):
    """Partition + order edges per core; build all per-core device arrays."""
    ei = inputs["edge_index"]
    row, col = ei[0].astype(np.int64), ei[1].astype(np.int64)
    eattr = inputs["edge_attr"][:, 0].astype(np.float32)
    emask = inputs["edge_mask"][:, 0].astype(np.float32)

    core = row // NSR
    lr = row - core * NSR
    win = lr // 128
    trow = (col // NSR) * NSP + (col % NSR)   # padded-table row of source node
    half = (trow >= HALF).astype(np.int64)

    # counts[c, half, w]
    key = (core * 2 + half) * NWIN + win
    counts = np.bincount(key, minlength=NCORES * 2 * NWIN).reshape(NCORES, 2, NWIN)
    quota = counts.max(axis=0)                      # [2, NWIN]
    quota = ((quota + 127) // 128) * 128

    half_len = [int(quota[hf].sum()) for hf in range(2)]
    half_pad = [((L + GCHUNK - 1) // GCHUNK) * GCHUNK for L in half_len]
    epad = half_pad[0] + half_pad[1]

    # section start offsets (same for every core)
    sec_start = np.zeros((2, NWIN), np.int64)
    off = 0
    for hf in range(2):
        off = half_pad[0] if hf == 1 else 0
        for w in range(NWIN):
            sec_start[hf, w] = off
            off += quota[hf, w]

    # chunk -> window map (128-edge chunks)
    nchunk = epad // 128
    chunk_win = np.full(nchunk, NWIN - 1, np.int64)
    chunk_half = np.zeros(nchunk, np.int64)
    for hf in range(2):
        for w in range(NWIN):
            s = sec_start[hf, w]
            for ch in range(int(quota[hf, w]) // 128):
                chunk_win[(s // 128) + ch] = w
        base = half_pad[0] if hf == 1 else 0
        lo = base // 128
        hi = (base + half_pad[hf]) // 128
        chunk_half[lo:hi] = hf

    per_core = []
    order = np.lexsort((lr, win, half, core))  # by core, half, win
    row_s, lr_s, win_s, half_s, trow_s = (a[order] for a in (row, lr, win, half, trow))
    ea_s, em_s, core_s = eattr[order], emask[order], core[order]

    ntile = epad // 128
    for c in range(NCORES):
        sel = core_s == c
        clr, cwin, chalf, ctrow = lr_s[sel], win_s[sel], half_s[sel], trow_s[sel]
        cea, cem = ea_s[sel], em_s[sel]

        ridx = np.zeros(epad, np.int64)
        cidx = np.zeros(epad, np.int64)
        srow = np.full(epad, -1.0, np.float32)
        emk = np.zeros(epad, np.float32)
        eat = np.zeros(epad, np.float32)

        # fill each (half, window) section
        pos = 0
        for hf in range(2):
            for w in range(NWIN):
                m = (chalf == hf) & (cwin == w)
                k = int(m.sum())
                s = int(sec_start[hf, w])
                ridx[s:s + k] = clr[m]
                cidx[s:s + k] = ctrow[m] - hf * HALF
                srow[s:s + k] = (clr[m] - 128 * w).astype(np.float32)
                emk[s:s + k] = cem[m]
                eat[s:s + k] = cea[m]
                pos += k

        per_core.append(dict(
            ridx=pack_idx16(ridx, epad),
            cidx=pack_idx16(cidx, epad),
            srow=np.ascontiguousarray(srow.reshape(-1, 128).T),   # [128, ntile]
            emask=np.ascontiguousarray(
                emk.reshape(-1, 128).T).astype(ml_dtypes.bfloat16),
            eattrT=eat.reshape(1, epad),
        ))

    meta = dict(epad=epad, nchunk=nchunk, chunk_win=chunk_win,
                chunk_half=chunk_half, half_pad=half_pad, ntile=ntile)
    return per_core, meta


def build_tables(inputs):
    """Padded-layout global tables (bf16 h, f32 x) + per-core slices."""
    h = inputs["h"].astype(np.float32)
    x = inputs["x"].astype(np.float32)
    ht = np.zeros((TPAD, H), ml_dtypes.bfloat16)
    xt = np.zeros((TPAD, 64), np.float32)
    for c in range(NCORES):
        ht[c * NSP:c * NSP + NSR] = h[c * NSR:(c + 1) * NSR].astype(ml_dtypes.bfloat16)
        xt[c * NSP:c * NSP + NSR, :3] = x[c * NSR:(c + 1) * NSR]
    return ht, xt


# ---------------------------------------------------------------- program

def build_program(meta, consts):
    epad = meta["epad"]
    ntile = meta["ntile"]           # 128-edge chunks
    nmtile = epad // ETILE          # 512-edge MLP tiles
    chunk_win = meta["chunk_win"]
    chunk_half = meta["chunk_half"]
    half_pad = meta["half_pad"]

    nc = bass.Bass()
    inp = {}

    def din(name, shape, dtype):
        inp[name] = nc.dram_tensor(name, shape, dtype, kind="ExternalInput")
        return inp[name]

    # tables
    tb_h_own = din("tb_h_own", [NSP, H], BF16)
    tb_h_lo = din("tb_h_lo", [HALF, H], BF16)
    tb_h_hi = din("tb_h_hi", [TPAD - HALF, H], BF16)
    tb_x_own = din("tb_x_own", [NSP, 64], F32)
    tb_x_lo = din("tb_x_lo", [HALF, 64], F32)
    tb_x_hi = din("tb_x_hi", [TPAD - HALF, 64], F32)
    # per-edge
    ridx = din("ridx", [128, epad // 16], mybir.dt.int16)
    cidx = din("cidx", [128, epad // 16], mybir.dt.int16)
    srow_in = din("srow", [128, ntile], F32)
    emask_in = din("emask", [128, ntile], BF16)
    eattrT = din("eattrT", [1, epad], F32)
    # node state
    hT0 = din("hT0", [128, NSP], F32)
    xT0 = din("xT0", [4, NSP], F32)
    nmaskT = din("nmaskT", [1, NSP], F32)
    # weights
    wt = {}
    for nm, shape, dt in consts["weights_spec"]:
        wt[nm] = din(nm, shape, dt)

    # outputs
    hT_out = nc.dram_tensor("hT_out", [128, NSP], F32, kind="ExternalOutput")
    xT_out = nc.dram_tensor("xT_out", [4, NSP], F32, kind="ExternalOutput")

    # internal DRAM
    ea_hbm = nc.dram_tensor("ea_hbm", [2, epad], F32)
    cc_in = [nc.dram_tensor(f"cc_in{i}", [NSP, H], BF16) for i in range(2)]
    cc_out = [nc.dram_tensor(f"cc_out{i}", [TPAD, H], BF16, addr_space="Shared")
              for i in range(2)]

    rg = [list(range(NCORES))]

    with TileContext(nc) as tc:
        import contextlib
        ctx = contextlib.ExitStack()
        with ctx:
            persist = ctx.enter_context(tc.tile_pool(name="persist", bufs=1))
            gpool = ctx.enter_context(tc.tile_pool(name="gath", bufs=2))
            work = ctx.enter_context(tc.tile_pool(name="work", bufs=3))
            ps_mlp = ctx.enter_context(tc.tile_pool(name="psmlp", bufs=3, space="PSUM"))
            ps_small = ctx.enter_context(tc.tile_pool(name="pssm", bufs=2, space="PSUM"))
            ps_agg = ctx.enter_context(tc.tile_pool(name="psagg", bufs=2, space="PSUM"))

            nc.gpsimd.load_library(library_config.mlp)

            # ---------------- persistent tiles
            ident = persist.tile([128, 128], F32)
            make_identity(nc, ident[:])
            identb = persist.tile([128, 128], BF16)
            nc.vector.tensor_copy(identb[:], ident[:])
            iota_i = persist.tile([128, 128], mybir.dt.int32)
            nc.vector.iota(iota_i[:], pattern=[[1, 128]], base=0, channel_multiplier=0)
            iota_f = persist.tile([128, 128], F32)
            nc.vector.tensor_copy(iota_f[:], iota_i[:])
            ones3 = persist.tile([3, 1], F32)
            nc.vector.memset(ones3[:], 1.0)
            ones1 = persist.tile([1, 128], F32)
            nc.vector.memset(ones1[:], 1.0)

            srow_t = persist.tile([128, ntile], F32)
            nc.sync.dma_start(srow_t[:], srow_in[:, :])
            emask_t = persist.tile([128, ntile], BF16)
            nc.sync.dma_start(emask_t[:], emask_in[:, :])
            ridx_t = persist.tile([128, epad // 16], mybir.dt.int16)
            nc.sync.dma_start(ridx_t[:], ridx[:, :])
            cidx_t = persist.tile([128, epad // 16], mybir.dt.int16)
            nc.sync.dma_start(cidx_t[:], cidx[:, :])

            hT = persist.tile([128, NSP], F32)
            nc.sync.dma_start(hT[:], hT0[:, :])
            aggT = persist.tile([128, NSP], F32)
            cdiff = persist.tile([128, ntile, 3], F32)

            # node-mask broadcast [128, NSP] via K=1 matmul
            mask_b = persist.tile([128, NSP], F32)
            for j in range(NSP // 448 + 1):
                a, b = j * 448, min((j + 1) * 448, NSP)
                if a >= b:
                    break
                mps = ps_small.tile([128, 448], F32, space="PSUM", tag="maskps")
                nc.tensor.matmul(mps[:, :b - a], ones1[:], nmaskT[0:1, a:b].to_ap(),
                                 start=True, stop=True) if False else None
            # simpler: load nmask row then matmul from SBUF
            nmask_sb = persist.tile([1, NSP], F32)
            nc.sync.dma_start(nmask_sb[:], nmaskT[:, :])
            for j in range((NSP + 447) // 448):
                a, b = j * 448, min((j + 1) * 448, NSP)
                mps = ps_small.tile([128, 448], F32, space="PSUM", tag="maskps")
                nc.tensor.matmul(mps[:, :b - a], ones1[:], nmask_sb[0:1, a:b],
                                 start=True, stop=True)
                nc.vector.tensor_copy(mask_b[:, a:b], mps[:, :b - a])

            # eattr -> ea_hbm row 1
            nc.sync.dma_start(ea_hbm[1:2, :], eattrT[:, :])

            # gather chunk table: (start, half) for col gathers
            def gather_chunks():
                out = []
                for s in range(0, half_pad[0], GCHUNK):
                    out.append((s, 0))
                for s in range(half_pad[0], epad, GCHUNK):
                    out.append((s, 1))
                return out

            chunks = gather_chunks()
            qrr = [0]

            def qn():
                qrr[0] = (qrr[0] + 1) % 4
                return qrr[0]

            def gather(dst, table, idxt, s, elem, transpose):
                nc.gpsimd.dma_gather(
                    dst[:], table[:], idxt[:, s // 16:(s + GCHUNK) // 16],
                    GCHUNK, GCHUNK, elem, transpose=transpose, queue_num=qn())

            # ---------------- prep pass: radial + coord_diff
            for (s, hf) in chunks:
                gxr = gpool.tile([128, GCHUNK // 128, 64], F32, tag="gxr")
                gather(gxr, tb_x_own, ridx_t, s, 64, False)
                gxc = gpool.tile([128, GCHUNK // 128, 64], F32, tag="gxc")
                gather(gxc, tb_x_lo if hf == 0 else tb_x_hi, cidx_t, s, 64, False)
                for t in range(GCHUNK // ETILE):
                    c0 = (s + t * ETILE) // 128       # first 128-chunk index
                    j0 = t * (ETILE // 128)
                    dif = work.tile([128, 4, 3], F32, tag="dif")
                    nc.vector.tensor_sub(dif[:], gxr[:, j0:j0 + 4, 0:3],
                                         gxc[:, j0:j0 + 4, 0:3])
                    sq = work.tile([128, 4, 3], F32, tag="sq")
                    nc.vector.tensor_mul(sq[:], dif[:], dif[:])
                    rad = work.tile([128, 4], F32, tag="rad")
                    nc.vector.tensor_add(rad[:], sq[:, :, 0], sq[:, :, 1])
                    nc.vector.tensor_add(rad[:], rad[:], sq[:, :, 2])
                    nrm = work.tile([128, 4], F32, tag="nrm")
                    nc.scalar.activation(nrm[:], rad[:], AF.Sqrt, bias=EPS)
                    nc.vector.tensor_scalar_add(nrm[:], nrm[:], 1.0)
                    rcp = work.tile([128, 4], F32, tag="rcp")
                    nc.vector.reciprocal(rcp[:], nrm[:])
                    nc.vector.tensor_mul(cdiff[:, c0:c0 + 4, :], dif[:],
                                         rcp[:].to_broadcast([128, 4, 3]))
                    # radial^T -> ea_hbm row 0
                    rps = ps_small.tile([4, 128], F32, space="PSUM", tag="radps")
                    nc.tensor.transpose(rps[:], rad[:], ident[:, 0:4])
                    rsb = work.tile([4, 128], F32, tag="rsb")
                    nc.vector.tensor_copy(rsb[:], rps[:])
                    dst = ea_hbm[0, s + t * ETILE: s + (t + 1) * ETILE]
                    nc.sync.dma_start(dst.rearrange("(a b) -> a b", a=4), rsb[:])

            tc.strict_bb_all_engine_barrier()

            # ---------------- edge MLP + aggregation pass
            def edge_pass(layer, tb_own, tb_lo, tb_hi, w1h, w1c, w1e, b1, w2, b2,
                          attw, attb, is_coord, cw3):
                """Runs one edge-MLP pass; fills aggT (feature-major, f32)."""
                nc.vector.memset(aggT[:], 0.0)
                open_win = {}
                for (s, hf) in chunks:
                    ghr = gpool.tile([128, 1, GCHUNK], BF16, tag="ghr")
                    gather(ghr, tb_own, ridx_t, s, H, True)
                    ghc = gpool.tile([128, 1, GCHUNK], BF16, tag="ghc")
                    gather(ghc, tb_lo if hf == 0 else tb_hi, cidx_t, s, H, True)
                    ea = gpool.tile([2, GCHUNK], F32, tag="ea")
                    nc.sync.dma_start(ea[:], ea_hbm[:, s:s + GCHUNK])
                    eab = gpool.tile([2, GCHUNK], BF16, tag="eab")
                    nc.vector.tensor_copy(eab[:], ea[:])
                    for t in range(GCHUNK // ETILE):
                        e0 = s + t * ETILE
                        c0 = e0 // 128
                        sl = slice(t * ETILE, (t + 1) * ETILE)
                        # --- MLP: pre1^T = W1.T @ einp^T
                        p1 = ps_mlp.tile([128, ETILE], F32, space="PSUM", tag="p1")
                        nc.tensor.matmul(p1[:], w1h[:], ghr[:, 0, sl], start=True, stop=False)
                        nc.tensor.matmul(p1[:], w1c[:], ghc[:, 0, sl], start=False, stop=False)
                        nc.tensor.matmul(p1[:], w1e[:], eab[:, sl], start=False, stop=True)
                        a1 = work.tile([128, ETILE], BF16, tag="a1")
                        nc.scalar.activation(a1[:], p1[:], AF.Silu, bias=b1[:])
                        p2 = ps_mlp.tile([128, ETILE], F32, space="PSUM", tag="p2")
                        nc.tensor.matmul(p2[:], w2[:], a1[:], start=True, stop=True)
                        mij = work.tile([128, ETILE], BF16, tag="mij")
                        nc.scalar.activation(mij[:], p2[:], AF.Silu, bias=b2[:])
                        # --- attention / phi: K-sliced matmuls -> [128e, 4]
                        ap = ps_small.tile([128, 4], F32, space="PSUM", tag="attps")
                        for j in range(4):
                            nc.tensor.matmul(ap[:, j:j + 1], mij[:, j * 128:(j + 1) * 128],
                                             cw3[:] if is_coord else attw[:],
                                             start=True, stop=True)
                        att = work.tile([128, 4], BF16, tag="att")
                        if is_coord:
                            nc.scalar.activation(att[:], ap[:], AF.Identity, bias=0.0)
                        else:
                            nc.scalar.activation(att[:], ap[:], AF.Sigmoid, bias=attb)
                        nc.vector.tensor_mul(att[:], att[:], emask_t[:, c0:c0 + 4])
                        # --- ef edge-major
                        if is_coord:
                            ef = work.tile([128, 4, 3], BF16, tag="ef3")
                            nc.vector.tensor_mul(
                                ef[:], cdiff[:, c0:c0 + 4, :],
                                att[:].to_broadcast([128, 4, 3]))
                            efw = 3
                        else:
                            mt = ps_small.tile([128, 4, 128], BF16, space="PSUM", tag="mtps")
                            for j in range(4):
                                nc.tensor.transpose(mt[:, j, :], mij[:, j * 128:(j + 1) * 128],
                                                    identb[:])
                            ef = work.tile([128, 4, 128], BF16, tag="ef")
                            nc.vector.tensor_mul(ef[:], mt[:],
                                                 att[:].to_broadcast([128, 4, 128]))
                            efw = 128
                        # --- selection matrix + window matmuls
                        S = work.tile([128, 4, 128], BF16, tag="S")
                        nc.vector.tensor_tensor(
                            out=S[:], in0=srow_t[:, c0:c0 + 4].to_broadcast([128, 4, 128]),
                            in1=iota_f[:].rearrange("p f -> p 1 f").to_broadcast([128, 4, 128]),
                            op=mybir.AluOpType.is_equal)
                        for j in range(4):
                            ch = c0 + j
                            w = int(chunk_win[ch])
                            if w not in open_win:
                                open_win[w] = ps_agg.tile(
                                    [128 if not is_coord else 3, 128], F32,
                                    space="PSUM", tag="aggps")
                            first = chunk_is_first[ch]
                            nc.tensor.matmul(open_win[w][:], ef[:, j, :efw], S[:, j, :],
                                             start=first, stop=chunk_is_last[ch])
                            if chunk_is_last[ch]:
                                pw = open_win.pop(w)
                                nc.vector.tensor_add(
                                    aggT[:efw if is_coord else 128,
                                         w * 128:(w + 1) * 128],
                                    aggT[:efw if is_coord else 128,
                                         w * 128:(w + 1) * 128],
                                    pw[:])
                assert not open_win

            # chunk first/last flags per (half, window) run
            chunk_is_first = np.zeros(ntile, bool)
            chunk_is_last = np.zeros(ntile, bool)
            for ch in range(ntile):
                prev_same = ch > 0 and chunk_win[ch - 1] == chunk_win[ch] \
                    and chunk_half[ch - 1] == chunk_half[ch]
                next_same = ch + 1 < ntile and chunk_win[ch + 1] == chunk_win[ch] \
                    and chunk_half[ch + 1] == chunk_half[ch]
                chunk_is_first[ch] = not prev_same
                chunk_is_last[ch] = not next_same

            # ---------------- node MLP + table rebuild
            def node_pass(layer, w1h_n, w1a_n, b1_n, w2_n, b2_n, cc_in_t):
                for j in range((NSP + 511) // 512):
                    a, b = j * 512, min((j + 1) * 512, NSP)
                    hb = work.tile([128, 512], BF16, tag="hb")
                    nc.vector.tensor_copy(hb[:, :b - a], hT[:, a:b])
                    ab = work.tile([128, 512], BF16, tag="ab")
                    nc.vector.tensor_copy(ab[:, :b - a], aggT[:, a:b])
                    np1 = ps_mlp.tile([128, 512], F32, space="PSUM", tag="np1")
                    nc.tensor.matmul(np1[:, :b - a], w1h_n[:], hb[:, :b - a],
                                     start=True, stop=False)
                    nc.tensor.matmul(np1[:, :b - a], w1a_n[:], ab[:, :b - a],
                                     start=False, stop=True)
                    na = work.tile([128, 512], BF16, tag="na")
                    nc.scalar.activation(na[:, :b - a], np1[:, :b - a], AF.Silu,
                                         bias=b1_n[:])
                    np2 = ps_mlp.tile([128, 512], F32, space="PSUM", tag="np2")
                    nc.tensor.matmul(np2[:, :b - a], w2_n[:], na[:, :b - a],
                                     start=True, stop=True)
                    dl = work.tile([128, 512], F32, tag="dl")
                    nc.scalar.activation(dl[:, :b - a], np2[:, :b - a], AF.Identity,
                                         bias=b2_n[:])
                    nc.vector.tensor_add(hT[:, a:b], hT[:, a:b], dl[:, :b - a])
                    nc.vector.tensor_mul(hT[:, a:b], hT[:, a:b], mask_b[:, a:b])
                # rebuild row-major bf16 table
                for j in range(NWIN):
                    a = j * 128
                    tp = ps_small.tile([128, 128], F32, space="PSUM", tag="tbps")
                    nc.tensor.transpose(tp[:], hT[:, a:a + 128], ident[:])
                    tsb = work.tile([128, 128], BF16, tag="tsb")
                    nc.vector.tensor_copy(tsb[:], tp[:])
                    nc.sync.dma_start(cc_in_t[a:a + 128, :], tsb[:])

            # ================ layer 0 and 1
            for li in range(2):
                if li == 0:
                    tbo, tbl, tbh = tb_h_own, tb_h_lo, tb_h_hi
                else:
                    tbo = cc_in[li - 1]
                    tbl = cc_out[li - 1][0:HALF, :]
                    tbh = cc_out[li - 1][HALF:TPAD, :]
                edge_pass(li, tbo, tbl, tbh,
                          wt[f"eW1h_{li}"], wt[f"eW1c_{li}"], wt[f"eW1e_{li}"],
                          wt[f"eb1_{li}"], wt[f"eW2_{li}"], wt[f"eb2_{li}"],
                          wt[f"aW_{li}"], consts["ab"][li], False, None)
                node_pass(li, wt[f"nW1h_{li}"], wt[f"nW1a_{li}"], wt[f"nb1_{li}"],
                          wt[f"nW2_{li}"], wt[f"nb2_{li}"], cc_in[li])
                tc.strict_bb_all_engine_barrier()
                nc.gpsimd.collective_compute(
                    "AllGather", mybir.AluOpType.bypass, replica_groups=rg,
                    ins=[cc_in[li][:]], outs=[cc_out[li][:]])
                tc.strict_bb_all_engine_barrier()

            # final h mask + output
            for j in range((NSP + 511) // 512):
                a, b = j * 512, min((j + 1) * 512, NSP)
                nc.vector.tensor_mul(hT[:, a:b], hT[:, a:b], mask_b[:, a:b])
            nc.sync.dma_start(hT_out[:, :], hT[:])

            # ================ coordinate pass
            edge_pass(2, cc_in[1], cc_out[1][0:HALF, :], cc_out[1][HALF:TPAD, :],
                      wt["cW1h"], wt["cW1c"], wt["cW1e"], wt["cb1"],
                      wt["cW2"], wt["cb2"], None, 0.0, True, wt["cW3"])
            # x update per window
            for w in range(NWIN):
                a = w * 128
                xin = work.tile([3, 128], F32, tag="xin")
                nc.sync.dma_start(xin[:], xT0[0:3, a:a + 128])
                nc.vector.tensor_add(xin[:], xin[:], aggT[0:3, a:a + 128])
                nc.vector.tensor_mul(xin[:], xin[:], mask_b[0:3, a:a + 128])
                nc.sync.dma_start(xT_out[0:3, a:a + 128], xin[:])

    return nc


# ---------------------------------------------------------------- weights

def weight_arrays(inputs):
    """CPU weight prep: bf16 casts, splits, folded normalizations."""
    bf = ml_dtypes.bfloat16
    spec, arrs = [], {}

    def add(nm, arr, dt):
        spec.append((nm, list(arr.shape), dt))
        arrs[nm] = arr

    for i in range(2):
        w1 = inputs["gcl_e_w1"][i].astype(np.float32)      # [258,128]
        add(f"eW1h_{i}", w1[0:128].astype(bf), BF16)
        add(f"eW1c_{i}", w1[128:256].astype(bf), BF16)
        add(f"eW1e_{i}", w1[256:258].astype(bf), BF16)
        add(f"eb1_{i}", inputs["gcl_e_b1"][i].reshape(128, 1).astype(np.float32), F32)
        add(f"eW2_{i}", inputs["gcl_e_w2"][i].astype(bf), BF16)
        add(f"eb2_{i}", inputs["gcl_e_b2"][i].reshape(128, 1).astype(np.float32), F32)
        add(f"aW_{i}", inputs["gcl_a_w"][i].astype(bf), BF16)   # [128,1]
        nw1 = inputs["gcl_n_w1"][i].astype(np.float32)      # [256,128]
        add(f"nW1h_{i}", nw1[0:128].astype(bf), BF16)
        add(f"nW1a_{i}", (nw1[128:256] / NORM_FACTOR).astype(bf), BF16)
        add(f"nb1_{i}", inputs["gcl_n_b1"][i].reshape(128, 1).astype(np.float32), F32)
        add(f"nW2_{i}", inputs["gcl_n_w2"][i].astype(bf), BF16)
        add(f"nb2_{i}", inputs["gcl_n_b2"][i].reshape(128, 1).astype(np.float32), F32)
    cw1 = inputs["c_w1"].astype(np.float32)
    add("cW1h", cw1[0:128].astype(bf), BF16)
    add("cW1c", cw1[128:256].astype(bf), BF16)
    add("cW1e", cw1[256:258].astype(bf), BF16)
    add("cb1", inputs["c_b1"].reshape(128, 1).astype(np.float32), F32)
    add("cW2", inputs["c_w2"].astype(bf), BF16)
    add("cb2", inputs["c_b2"].reshape(128, 1).astype(np.float32), F32)
    add("cW3", (inputs["c_w3"] / NORM_FACTOR).astype(bf), BF16)
    ab = [float(inputs["gcl_a_b"][i][0]) for i in range(2)]
    return spec, arrs, ab


# ---------------------------------------------------------------- entry

def kernel(**inputs):
    per_core, meta = shard(inputs)
    ht, xt = build_tables(inputs)
    spec, warrs, ab = weight_arrays(inputs)

    consts = dict(weights_spec=spec, ab=ab)
    nc = build_program(meta, consts)
    lower_extended_insts(nc)
    split_excess_waits(nc)

    h = inputs["h"].astype(np.float32)
    x = inputs["x"].astype(np.float32)
    nmask = inputs["node_mask"][:, 0].astype(np.float32)

    in_maps = []
    for c in range(NCORES):
        hT0 = np.zeros((128, NSP), np.float32)
        hT0[:, :NSR] = h[c * NSR:(c + 1) * NSR].T
        xT0 = np.zeros((4, NSP), np.float32)
        xT0[:3, :NSR] = x[c * NSR:(c + 1) * NSR].T
        nmT = np.zeros((1, NSP), np.float32)
        nmT[0, :NSR] = nmask[c * NSR:(c + 1) * NSR]
        m = dict(
            tb_h_own=np.ascontiguousarray(ht[c * NSP:(c + 1) * NSP]),
            tb_h_lo=np.ascontiguousarray(ht[:HALF]),
            tb_h_hi=np.ascontiguousarray(ht[HALF:]),
            tb_x_own=np.ascontiguousarray(xt[c * NSP:(c + 1) * NSP]),
            tb_x_lo=np.ascontiguousarray(xt[:HALF]),
            tb_x_hi=np.ascontiguousarray(xt[HALF:]),
            hT0=hT0, xT0=xT0, nmaskT=nmT,
            **per_core[c], **warrs)
        in_maps.append(m)

    res = run_bass_kernel_spmd(nc, in_maps, core_ids=list(range(NCORES)))

    h_out = np.zeros((N, H), np.float32)
    x_out = np.zeros((N, 3), np.float32)
    for c in range(NCORES):
        r = res.results[c]
        h_out[c * NSR:(c + 1) * NSR] = r["hT_out"][:, :NSR].T
        x_out[c * NSR:(c + 1) * NSR] = r["xT_out"][:3, :NSR].T
    return (h_out, x_out)


# revision 14
# speedup vs baseline: 1.1061x; 1.1061x over previous
"""EquivariantBlock (EGNN) Trainium2 kernel — 8-core SPMD.

Sharding: nodes by destination row (6250/core, padded to 6272). Each core owns
the edges whose aggregation target (row) lands in its slice. CPU-side prep
sorts each core's edges by (col-half, row-window) and pads per-(half,window)
sections to shared quotas so one compiled program serves all 8 cores.

Device pipeline per core:
  prep : gather x[row],x[col] (fp32, edge-major) -> radial, coord_diff,
         radial^T -> ea table in HBM
  GCL x2: gather h[row],h[col] (bf16, transposed -> feature-major), edge MLP
         on TensorE, attention via K-sliced matmuls, ef -> edge-major via PE
         transpose, segment-sum via selection-matrix matmuls into per-window
         PSUM accumulated into agg^T (feature-major) in SBUF, node MLP,
         h residual update, rebuild bf16 h-table, AllGather across cores
  coord: same edge MLP with coord weights, phi * coord_diff * mask
         aggregated the same way, x update.
"""

import os
import numpy as np
import ml_dtypes

import concourse.bass as bass
import concourse.mybir as mybir
from concourse.tile import TileContext
from concourse.bass_utils import run_bass_kernel_spmd
from concourse.masks import make_identity
from concourse import library_config
from concourse.library_overlay import lower_extended_insts

BF16 = mybir.dt.bfloat16
F32 = mybir.dt.float32
AF = mybir.ActivationFunctionType

N = 50000
E = 800000
H = 128
NCORES = 8
NSR = 6250          # real rows per core
NSP = 6272          # padded rows per core (49 * 128)
NWIN = NSP // 128   # 49 windows
TPAD = NCORES * NSP  # 50176 padded global table rows
HALF = 32768        # int16-safe table split
EPS = 1e-8
NORM_FACTOR = 100.0
GCHUNK = 2048       # idxs per dma_gather
ETILE = 512         # edges per MLP tile


def _wait_split_counter():
    c = [0]
    def fresh():
        c[0] += 1
        return f"WSPLIT-{c[0]}"
    return fresh


def split_excess_waits(nc, max_keep=1):
    """This container's walrus rejects instructions with >2 sem waits
    ("Too many sync wait commands"). Hoist extras onto same-engine nops."""
    fresh = _wait_split_counter()
    total = 0
    for fn in nc.m.functions:
        for bb in fn.blocks:
            out, changed = [], False
            for inst in bb.instructions:
                si = inst.sync_info
                if si is not None and si.on_wait and len(si.on_wait) > max_keep \
                        and inst.engine != mybir.EngineType.Unassigned:
                    hoist = [w for w in si.on_wait
                             if w.sync_type == "semaphore" and w.wait_reg is None]
                    keep = [w for w in si.on_wait
                            if not (w.sync_type == "semaphore" and w.wait_reg is None)]
                    while len(hoist) + len(keep) > max_keep and hoist:
                        w = hoist.pop(0)
                        nop = mybir.InstNoOp(name=fresh(), ins=[], outs=[])
                        nop.engine = inst.engine
                        nop.sync_info = mybir.SyncInfo(on_wait=[w], on_update=[])
                        out.append(nop)
                        total += 1
                        changed = True
                    si.on_wait = hoist + keep
                out.append(inst)
            if changed:
                bb.instructions = out
    return total


def pack_idx16(idx, epad):
    """int16 idx tile [128, epad/16]; position i -> (i%16, i//16), replicated
    across the 8 groups of 16 partitions."""
    buf = np.zeros(epad, np.int16)
    buf[:idx.shape[0]] = idx.astype(np.int16)
    t = buf.reshape(-1, 16).T
    return np.ascontiguousarray(np.tile(t, (8, 1)))


# ---------------------------------------------------------------- CPU shard

def shard(inputs):
    """Partition + order edges per core; build all per-core device arrays."""
    ei = inputs["edge_index"]
    row, col = ei[0].astype(np.int64), ei[1].astype(np.int64)
    eattr = inputs["edge_attr"][:, 0].astype(np.float32)
    emask = inputs["edge_mask"][:, 0].astype(np.float32)

    core = row // NSR
    lr = row - core * NSR
    win = lr // 128
    trow = (col // NSR) * NSP + (col % NSR)   # padded-table row of source node
    half = (trow >= HALF).astype(np.int64)

    # counts[c, half, w]
    key = (core * 2 + half) * NWIN + win
    counts = np.bincount(key, minlength=NCORES * 2 * NWIN).reshape(NCORES, 2, NWIN)
    quota = counts.max(axis=0)                      # [2, NWIN]
    quota = ((quota + 127) // 128) * 128

    half_len = [int(quota[hf].sum()) for hf in range(2)]
    half_pad = [((L + GCHUNK - 1) // GCHUNK) * GCHUNK for L in half_len]
    epad = half_pad[0] + half_pad[1]

    # section start offsets (same for every core)
    sec_start = np.zeros((2, NWIN), np.int64)
    off = 0
    for hf in range(2):
        off = half_pad[0] if hf == 1 else 0
        for w in range(NWIN):
            sec_start[hf, w] = off
            off += quota[hf, w]

    # chunk -> window map (128-edge chunks)
    nchunk = epad // 128
    chunk_win = np.full(nchunk, NWIN - 1, np.int64)
    chunk_half = np.zeros(nchunk, np.int64)
    for hf in range(2):
        for w in range(NWIN):
            s = sec_start[hf, w]
            for ch in range(int(quota[hf, w]) // 128):
                chunk_win[(s // 128) + ch] = w
        base = half_pad[0] if hf == 1 else 0
        lo = base // 128
        hi = (base + half_pad[hf]) // 128
        chunk_half[lo:hi] = hf

    per_core = []
    order = np.lexsort((lr, win, half, core))  # by core, half, win
    row_s, lr_s, win_s, half_s, trow_s = (a[order] for a in (row, lr, win, half, trow))
    ea_s, em_s, core_s = eattr[order], emask[order], core[order]

    ntile = epad // 128
    for c in range(NCORES):
        sel = core_s == c
        clr, cwin, chalf, ctrow = lr_s[sel], win_s[sel], half_s[sel], trow_s[sel]
        cea, cem = ea_s[sel], em_s[sel]

        ridx = np.zeros(epad, np.int64)
        cidx = np.zeros(epad, np.int64)
        srow = np.full(epad, -1.0, np.float32)
        emk = np.zeros(epad, np.float32)
        eat = np.zeros(epad, np.float32)

        # fill each (half, window) section
        pos = 0
        for hf in range(2):
            for w in range(NWIN):
                m = (chalf == hf) & (cwin == w)
                k = int(m.sum())
                s = int(sec_start[hf, w])
                ridx[s:s + k] = clr[m]
                cidx[s:s + k] = ctrow[m] - hf * HALF
                srow[s:s + k] = (clr[m] - 128 * w).astype(np.float32)
                emk[s:s + k] = cem[m]
                eat[s:s + k] = cea[m]
                pos += k

        per_core.append(dict(
            ridx=pack_idx16(ridx, epad),
            cidx=pack_idx16(cidx, epad),
            srow=np.ascontiguousarray(srow.reshape(-1, 128).T),   # [128, ntile]
            emask=np.ascontiguousarray(
                emk.reshape(-1, 128).T).astype(ml_dtypes.bfloat16),
            eattrT=eat.reshape(1, epad),
        ))

    meta = dict(epad=epad, nchunk=nchunk, chunk_win=chunk_win,
                chunk_half=chunk_half, half_pad=half_pad, ntile=ntile)
    return per_core, meta


def build_tables(inputs):
    """Padded-layout global tables (bf16 h, f32 x) + per-core slices."""
    h = inputs["h"].astype(np.float32)
    x = inputs["x"].astype(np.float32)
    ht = np.zeros((TPAD, H), ml_dtypes.bfloat16)
    xt = np.zeros((TPAD, 64), np.float32)
    for c in range(NCORES):
        ht[c * NSP:c * NSP + NSR] = h[c * NSR:(c + 1) * NSR].astype(ml_dtypes.bfloat16)
        xt[c * NSP:c * NSP + NSR, :3] = x[c * NSR:(c + 1) * NSR]
    return ht, xt


# ---------------------------------------------------------------- program

def build_program(meta, consts):
    PH = int(os.environ.get("KPHASES", "9"))  # 1=prep 2=+L1edge 3=+node 4=+AG 5=+L2 6=+coord
    epad = meta["epad"]
    ntile = meta["ntile"]           # number of 128-edge chunks
    chunk_win = meta["chunk_win"]
    chunk_half = meta["chunk_half"]
    half_pad = meta["half_pad"]

    # chunk first/last flags per (half, window) run
    chunk_is_first = np.zeros(ntile, bool)
    chunk_is_last = np.zeros(ntile, bool)
    for ch in range(ntile):
        prev_same = ch > 0 and chunk_win[ch - 1] == chunk_win[ch] \
            and chunk_half[ch - 1] == chunk_half[ch]
        next_same = ch + 1 < ntile and chunk_win[ch + 1] == chunk_win[ch] \
            and chunk_half[ch + 1] == chunk_half[ch]
        chunk_is_first[ch] = not prev_same
        chunk_is_last[ch] = not next_same

    nc = bass.Bass()
    inp = {}

    def din(name, shape, dtype):
        inp[name] = nc.dram_tensor(name, shape, dtype, kind="ExternalInput")
        return inp[name]

    tb_h_own = din("tb_h_own", [NSP, H], BF16)
    tb_h_lo = din("tb_h_lo", [HALF, H], BF16)
    tb_h_hi = din("tb_h_hi", [TPAD - HALF, H], BF16)
    tb_x_own = din("tb_x_own", [NSP, 64], F32)
    tb_x_lo = din("tb_x_lo", [HALF, 64], F32)
    tb_x_hi = din("tb_x_hi", [TPAD - HALF, 64], F32)
    ridx = din("ridx", [128, epad // 16], mybir.dt.int16)
    cidx = din("cidx", [128, epad // 16], mybir.dt.int16)
    srow_in = din("srow", [128, ntile], F32)
    emask_in = din("emask", [128, ntile], BF16)
    eattrT = din("eattrT", [1, epad], F32)
    hT0 = din("hT0", [128, NSP], F32)
    xT0 = din("xT0", [4, NSP], F32)
    nmaskT = din("nmaskT", [1, NSP], F32)
    wspec = consts["weights_spec"]
    for nm, shape, dt in wspec:
        din(nm, shape, dt)

    hT_out = nc.dram_tensor("hT_out", [128, NSP], F32, kind="ExternalOutput")
    xT_out = nc.dram_tensor("xT_out", [4, NSP], F32, kind="ExternalOutput")

    ea_hbm = nc.dram_tensor("ea_hbm", [2, epad], F32)
    cc_in = [nc.dram_tensor(f"cc_in{i}", [NSP, H], BF16) for i in range(2)]
    cc_out = [nc.dram_tensor(f"cc_out{i}", [TPAD, H], BF16, addr_space="Shared")
              for i in range(2)]
    rg = [list(range(NCORES))]

    import contextlib
    with TileContext(nc) as tc, contextlib.ExitStack() as ctx:
        persist = ctx.enter_context(tc.tile_pool(name="persist", bufs=1))
        gpool = ctx.enter_context(tc.tile_pool(name="gath", bufs=2))
        work = ctx.enter_context(tc.tile_pool(name="work", bufs=3))
        ps_mlp = ctx.enter_context(tc.tile_pool(name="psmlp", bufs=2, space="PSUM"))
        ps_small = ctx.enter_context(tc.tile_pool(name="pssm", bufs=2, space="PSUM"))
        ps_agg = ctx.enter_context(tc.tile_pool(name="psagg", bufs=2, space="PSUM"))

        # ---- persistent constants (gpsimd 'standard' library ops first)
        ident = persist.tile([128, 128], F32)
        make_identity(nc, ident[:])
        identb = persist.tile([128, 128], BF16)
        nc.vector.tensor_copy(identb[:], ident[:])
        iota_i = persist.tile([128, 128], mybir.dt.int32)
        nc.gpsimd.iota(iota_i[:], pattern=[[1, 128]], base=0, channel_multiplier=0)
        iota_f = persist.tile([128, 128], F32)
        nc.vector.tensor_copy(iota_f[:], iota_i[:])
        ones1 = persist.tile([1, 128], F32)
        nc.vector.memset(ones1[:], 1.0)
        tc.strict_bb_all_engine_barrier()
        nc.gpsimd.load_library(library_config.mlp)
        eps_t = persist.tile([128, 1], F32)
        nc.vector.memset(eps_t[:], EPS)
        ab_t = []
        for i in range(2):
            t = persist.tile([128, 1], F32, tag=f"ab{i}", name=f"ab{i}")
            nc.vector.memset(t[:], consts["ab"][i])
            ab_t.append(t)

        # ---- weights into SBUF
        wsb = {}
        for nm, shape, dt in wspec:
            t = persist.tile(list(shape), dt, tag=f"w_{nm}", name=f"w_{nm}")
            nc.sync.dma_start(t[:], inp[nm][:, :] if len(shape) == 2 else inp[nm][:])
            wsb[nm] = t

        srow_t = persist.tile([128, ntile], F32)
        nc.sync.dma_start(srow_t[:], srow_in[:, :])
        emask_t = persist.tile([128, ntile], BF16)
        nc.sync.dma_start(emask_t[:], emask_in[:, :])
        ridx_t = persist.tile([128, epad // 16], mybir.dt.int16)
        nc.sync.dma_start(ridx_t[:], ridx[:, :])
        cidx_t = persist.tile([128, epad // 16], mybir.dt.int16)
        nc.sync.dma_start(cidx_t[:], cidx[:, :])

        hT = persist.tile([128, NSP], F32)
        nc.sync.dma_start(hT[:], hT0[:, :])
        aggT = persist.tile([128, NSP], F32)
        cdiff = persist.tile([128, ntile, 3], F32)

        # node-mask broadcast [128, NSP] via K=1 matmuls
        mask_b = persist.tile([128, NSP], F32)
        for j in range((NSP + 511) // 512):
            a, b = j * 512, min((j + 1) * 512, NSP)
            nmk = work.tile([1, 512], F32, tag="nmk")
            nc.sync.dma_start(nmk[:, :b - a], nmaskT[:, a:b])
            mps = ps_small.tile([128, 4, 128], F32, space="PSUM", tag="smps")
            mview = mps[:].rearrange("p a f -> p (a f)")
            nc.tensor.matmul(mview[:, :b - a], ones1[:], nmk[0:1, :b - a],
                             start=True, stop=True)
            nc.vector.tensor_copy(mask_b[:, a:b], mview[:, :b - a])

        nc.sync.dma_start(ea_hbm[1:2, :], eattrT[:, :])

        def gather_chunks():
            out = []
            for s in range(0, half_pad[0], GCHUNK):
                out.append((s, 0))
            for s in range(half_pad[0], epad, GCHUNK):
                out.append((s, 1))
            return out

        chunks = gather_chunks()
        def qn():
            return 0

        greg = nc.gpsimd.to_reg(GCHUNK)

        def gather(dst, table_ap, idxt, s, elem, transpose):
            nc.gpsimd.dma_gather(
                dst[:], table_ap, idxt[:, s // 16:(s + GCHUNK) // 16],
                GCHUNK, greg, elem, transpose=transpose, queue_num=qn(),
                single_packet=False)

        # ---------------- prep pass: radial + coord_diff
        for (s, hf) in (chunks if PH >= 1 else ([] if PH == 0 else [])):
            gxr = gpool.tile([128, GCHUNK // 128, 64], F32, tag="ghr")
            gather(gxr, tb_x_own[:, :], ridx_t, s, 64, False)
            gxc = gpool.tile([128, GCHUNK // 128, 64], F32, tag="ghc")
            gather(gxc, (tb_x_lo if hf == 0 else tb_x_hi)[:, :], cidx_t, s, 64, False)
            for t in range(GCHUNK // ETILE if os.environ.get("KPREPMATH", "1") == "1" else 0):
                c0 = (s + t * ETILE) // 128
                j0 = t * (ETILE // 128)
                dif = work.tile([128, 4, 3], F32, tag="dif")
                nc.vector.tensor_sub(dif[:], gxr[:, j0:j0 + 4, 0:3],
                                     gxc[:, j0:j0 + 4, 0:3])
                sq = work.tile([128, 4, 3], F32, tag="sq")
                nc.vector.tensor_mul(sq[:], dif[:], dif[:])
                rad = work.tile([128, 4], F32, tag="rad")
                nc.vector.tensor_add(rad[:], sq[:, :, 0], sq[:, :, 1])
                nc.vector.tensor_add(rad[:], rad[:], sq[:, :, 2])
                nrm = work.tile([128, 4], F32, tag="nrm")
                nc.scalar.activation(nrm[:], rad[:], AF.Sqrt, bias=eps_t[:])
                nc.vector.tensor_scalar_add(nrm[:], nrm[:], 1.0)
                rcp = work.tile([128, 4], F32, tag="rcp")
                nc.vector.reciprocal(rcp[:], nrm[:])
                nc.vector.tensor_mul(cdiff[:, c0:c0 + 4, :], dif[:],
                                     rcp[:].to_broadcast([128, 4, 3]))
                rps = ps_small.tile([128, 4, 128], F32, space="PSUM", tag="smps")
                rview = rps[:].rearrange("p a f -> p (a f)")
                nc.tensor.transpose(rview[0:4, 0:128], rad[:], ident[:])
                rsb = work.tile([4, 128], F32, tag="rsb")
                nc.vector.tensor_copy(rsb[:], rview[0:4, 0:128])
                dst = ea_hbm[0:1, s + t * ETILE: s + (t + 1) * ETILE]
                nc.sync.dma_start(dst.rearrange("o (a b) -> (o a) b", a=4), rsb[:])

        tc.strict_bb_all_engine_barrier()

        # ---------------- edge MLP + aggregation pass
        def edge_pass(tb_own, tb_lo, tb_hi, w1h, w1c, w1e, b1, w2, b2,
                      attw, attb, is_coord):
            nc.vector.memset(aggT[:], 0.0)
            open_win = {}
            for (s, hf) in chunks:
                ghr = gpool.tile([128, 1, GCHUNK], BF16, tag="ghr")
                gather(ghr, tb_own, ridx_t, s, H, True)
                ghc = gpool.tile([128, 1, GCHUNK], BF16, tag="ghc")
                gather(ghc, tb_lo if hf == 0 else tb_hi, cidx_t, s, H, True)
                eab = gpool.tile([2, GCHUNK], BF16, tag="eab")
                nc.gpsimd.dma_start(eab[:], ea_hbm[:, s:s + GCHUNK])
                for t in range(GCHUNK // ETILE):
                    e0 = s + t * ETILE
                    c0 = e0 // 128
                    sl = slice(t * ETILE, (t + 1) * ETILE)
                    p1 = ps_mlp.tile([128, ETILE], F32, space="PSUM", tag="mmps")
                    nc.tensor.matmul(p1[:], w1h[:], ghr[:, 0, sl], start=True, stop=False)
                    nc.tensor.matmul(p1[:], w1c[:], ghc[:, 0, sl], start=False, stop=False)
                    nc.tensor.matmul(p1[:], w1e[:], eab[:, sl], start=False, stop=True)
                    a1 = work.tile([128, ETILE], BF16, tag="a1")
                    nc.scalar.activation(a1[:], p1[:], AF.Silu, bias=b1[:])
                    p2 = ps_mlp.tile([128, ETILE], F32, space="PSUM", tag="mmps")
                    nc.tensor.matmul(p2[:], w2[:], a1[:], start=True, stop=True)
                    mij = work.tile([128, ETILE], BF16, tag="mij")
                    nc.scalar.activation(mij[:], p2[:], AF.Silu, bias=b2[:])
                    ap = ps_small.tile([128, 4, 128], F32, space="PSUM", tag="smps")
                    for j in range(4):
                        nc.tensor.matmul(ap[:, j, 0:1], mij[:, j * 128:(j + 1) * 128],
                                         attw[:], start=True, stop=True)
                    att = work.tile([128, 4], BF16, tag="att")
                    if is_coord:
                        nc.scalar.activation(att[:], ap[:, :, 0], AF.Identity, bias=0.0)
                    else:
                        nc.scalar.activation(att[:], ap[:, :, 0], AF.Sigmoid, bias=attb[:])
                    nc.vector.tensor_mul(att[:], att[:], emask_t[:, c0:c0 + 4])
                    if is_coord:
                        ef = work.tile([128, 4, 3], BF16, tag="ef")
                        nc.vector.tensor_mul(
                            ef[:], cdiff[:, c0:c0 + 4, :],
                            att[:].to_broadcast([128, 4, 3]))
                        efw = 3
                    else:
                        mt = ps_small.tile([128, 4, 128], BF16, space="PSUM", tag="smpsb")
                        for j in range(4):
                            nc.tensor.transpose(mt[:, j, :], mij[:, j * 128:(j + 1) * 128],
                                                identb[:])
                        ef = work.tile([128, 4, 128], BF16, tag="ef")
                        nc.vector.tensor_mul(ef[:], mt[:],
                                             att[:].to_broadcast([128, 4, 128]))
                        efw = 128
                    S = work.tile([128, 4, 128], BF16, tag="S")
                    for j in range(4):
                        nc.vector.tensor_tensor(
                            out=S[:, j, :],
                            in0=srow_t[:, c0 + j:c0 + j + 1].to_broadcast([128, 128]),
                            in1=iota_f[:],
                            op=mybir.AluOpType.is_equal)
                    for j in range(4):
                        ch = c0 + j
                        w = int(chunk_win[ch])
                        key = (w, int(chunk_half[ch]))
                        if key not in open_win:
                            open_win[key] = ps_agg.tile([128, 128], F32, space="PSUM",
                                                        tag="aggps", name="aggwin")
                        nc.tensor.matmul(
                            open_win[key][0:efw if is_coord else 128, :],
                            ef[:, j, :], S[:, j, :],
                            start=bool(chunk_is_first[ch]),
                            stop=bool(chunk_is_last[ch]))
                        if chunk_is_last[ch]:
                            pw = open_win.pop(key)
                            rw = efw if is_coord else 128
                            nc.vector.tensor_add(
                                aggT[0:rw, w * 128:(w + 1) * 128],
                                aggT[0:rw, w * 128:(w + 1) * 128],
                                pw[0:rw, :])
            assert not open_win

        # ---------------- node MLP + table rebuild
        def node_pass(w1h_n, w1a_n, b1_n, w2_n, b2_n, cc_in_t):
            for j in range((NSP + 511) // 512):
                a, b = j * 512, min((j + 1) * 512, NSP)
                hb = work.tile([128, 512], BF16, tag="hb")
                nc.vector.tensor_copy(hb[:, :b - a], hT[:, a:b])
                ab = work.tile([128, 512], BF16, tag="ab")
                nc.vector.tensor_copy(ab[:, :b - a], aggT[:, a:b])
                np1 = ps_mlp.tile([128, 512], F32, space="PSUM", tag="mmps")
                nc.tensor.matmul(np1[:, :b - a], w1h_n[:], hb[:, :b - a],
                                 start=True, stop=False)
                nc.tensor.matmul(np1[:, :b - a], w1a_n[:], ab[:, :b - a],
                                 start=False, stop=True)
                na = work.tile([128, 512], BF16, tag="na")
                nc.scalar.activation(na[:, :b - a], np1[:, :b - a], AF.Silu,
                                     bias=b1_n[:])
                np2 = ps_mlp.tile([128, 512], F32, space="PSUM", tag="mmps")
                nc.tensor.matmul(np2[:, :b - a], w2_n[:], na[:, :b - a],
                                 start=True, stop=True)
                dl = work.tile([128, 512], F32, tag="dl")
                nc.scalar.activation(dl[:, :b - a], np2[:, :b - a], AF.Identity,
                                     bias=b2_n[:])
                nc.vector.tensor_add(hT[:, a:b], hT[:, a:b], dl[:, :b - a])
                nc.vector.tensor_mul(hT[:, a:b], hT[:, a:b], mask_b[:, a:b])
            for j in range(NWIN):
                a = j * 128
                tp = ps_small.tile([128, 4, 128], F32, space="PSUM", tag="smps")
                tview = tp[:].rearrange("p a f -> p (a f)")
                nc.tensor.transpose(tview[:, 0:128], hT[:, a:a + 128], ident[:])
                tsb = work.tile([128, 128], BF16, tag="tsb")
                nc.vector.tensor_copy(tsb[:], tview[:, 0:128])
                nc.sync.dma_start(cc_in_t[a:a + 128, :], tsb[:])

        for li in range(2 if PH >= 5 else (1 if PH >= 2 else 0)):
            if li == 0:
                tbo, tbl, tbh = tb_h_own[:, :], tb_h_lo[:, :], tb_h_hi[:, :]
            else:
                tbo = cc_in[li - 1][:, :]
                tbl = cc_out[li - 1][0:HALF, :]
                tbh = cc_out[li - 1][HALF:TPAD, :]
            edge_pass(tbo, tbl, tbh,
                      wsb[f"eW1h_{li}"], wsb[f"eW1c_{li}"], wsb[f"eW1e_{li}"],
                      wsb[f"eb1_{li}"], wsb[f"eW2_{li}"], wsb[f"eb2_{li}"],
                      wsb[f"aW_{li}"], ab_t[li], False)
            if PH >= 3:
                node_pass(wsb[f"nW1h_{li}"], wsb[f"nW1a_{li}"], wsb[f"nb1_{li}"],
                          wsb[f"nW2_{li}"], wsb[f"nb2_{li}"], cc_in[li])
            tc.strict_bb_all_engine_barrier()
            if PH >= 4:
                nc.gpsimd.collective_compute(
                    "AllGather", mybir.AluOpType.bypass, replica_groups=rg,
                    ins=[cc_in[li][:]], outs=[cc_out[li][:]])
                tc.strict_bb_all_engine_barrier()

        for j in range((NSP + 511) // 512):
            a, b = j * 512, min((j + 1) * 512, NSP)
            nc.vector.tensor_mul(hT[:, a:b], hT[:, a:b], mask_b[:, a:b])
        nc.sync.dma_start(hT_out[:, :], hT[:])

        if PH >= 6:
            edge_pass(cc_in[1][:, :], cc_out[1][0:HALF, :], cc_out[1][HALF:TPAD, :],
                      wsb["cW1h"], wsb["cW1c"], wsb["cW1e"], wsb["cb1"],
                      wsb["cW2"], wsb["cb2"], wsb["cW3"], None, True)
        else:
            nc.vector.memset(aggT[:], 0.0)
        for w in range(NWIN):
            a = w * 128
            xin = work.tile([3, 128], F32, tag="xin")
            nc.sync.dma_start(xin[:], xT0[0:3, a:a + 128])
            nc.vector.tensor_add(xin[:], xin[:], aggT[0:3, a:a + 128])
            nc.vector.tensor_mul(xin[:], xin[:], mask_b[0:3, a:a + 128])
            nc.sync.dma_start(xT_out[0:3, a:a + 128], xin[:])

        # restore the default gpsimd library so the NEFF is re-executable
        tc.strict_bb_all_engine_barrier()
        nc.gpsimd.load_library(library_config.standard)

    return nc


# ---------------------------------------------------------------- weights

def weight_arrays(inputs):
    """CPU weight prep: bf16 casts, splits, folded normalizations."""
    bf = ml_dtypes.bfloat16
    spec, arrs = [], {}

    def add(nm, arr, dt):
        spec.append((nm, list(arr.shape), dt))
        arrs[nm] = arr

    for i in range(2):
        w1 = inputs["gcl_e_w1"][i].astype(np.float32)      # [258,128]
        add(f"eW1h_{i}", w1[0:128].astype(bf), BF16)
        add(f"eW1c_{i}", w1[128:256].astype(bf), BF16)
        add(f"eW1e_{i}", w1[256:258].astype(bf), BF16)
        add(f"eb1_{i}", inputs["gcl_e_b1"][i].reshape(128, 1).astype(np.float32), F32)
        add(f"eW2_{i}", inputs["gcl_e_w2"][i].astype(bf), BF16)
        add(f"eb2_{i}", inputs["gcl_e_b2"][i].reshape(128, 1).astype(np.float32), F32)
        add(f"aW_{i}", inputs["gcl_a_w"][i].astype(bf), BF16)   # [128,1]
        nw1 = inputs["gcl_n_w1"][i].astype(np.float32)      # [256,128]
        add(f"nW1h_{i}", nw1[0:128].astype(bf), BF16)
        add(f"nW1a_{i}", (nw1[128:256] / NORM_FACTOR).astype(bf), BF16)
        add(f"nb1_{i}", inputs["gcl_n_b1"][i].reshape(128, 1).astype(np.float32), F32)
        add(f"nW2_{i}", inputs["gcl_n_w2"][i].astype(bf), BF16)
        add(f"nb2_{i}", inputs["gcl_n_b2"][i].reshape(128, 1).astype(np.float32), F32)
    cw1 = inputs["c_w1"].astype(np.float32)
    add("cW1h", cw1[0:128].astype(bf), BF16)
    add("cW1c", cw1[128:256].astype(bf), BF16)
    add("cW1e", cw1[256:258].astype(bf), BF16)
    add("cb1", inputs["c_b1"].reshape(128, 1).astype(np.float32), F32)
    add("cW2", inputs["c_w2"].astype(bf), BF16)
    add("cb2", inputs["c_b2"].reshape(128, 1).astype(np.float32), F32)
    add("cW3", (inputs["c_w3"] / NORM_FACTOR).astype(bf), BF16)
    ab = [float(inputs["gcl_a_b"][i][0]) for i in range(2)]
    return spec, arrs, ab


# ---------------------------------------------------------------- entry

def kernel(**inputs):
    per_core, meta = shard(inputs)
    ht, xt = build_tables(inputs)
    spec, warrs, ab = weight_arrays(inputs)

    consts = dict(weights_spec=spec, ab=ab)
    nc = build_program(meta, consts)
    lower_extended_insts(nc)
    split_excess_waits(nc)

    h = inputs["h"].astype(np.float32)
    x = inputs["x"].astype(np.float32)
    nmask = inputs["node_mask"][:, 0].astype(np.float32)

    in_maps = []
    for c in range(NCORES):
        hT0 = np.zeros((128, NSP), np.float32)
        hT0[:, :NSR] = h[c * NSR:(c + 1) * NSR].T
        xT0 = np.zeros((4, NSP), np.float32)
        xT0[:3, :NSR] = x[c * NSR:(c + 1) * NSR].T
        nmT = np.zeros((1, NSP), np.float32)
        nmT[0, :NSR] = nmask[c * NSR:(c + 1) * NSR]
        m = dict(
            tb_h_own=np.ascontiguousarray(ht[c * NSP:(c + 1) * NSP]),
            tb_h_lo=np.ascontiguousarray(ht[:HALF]),
            tb_h_hi=np.ascontiguousarray(ht[HALF:]),
            tb_x_own=np.ascontiguousarray(xt[c * NSP:(c + 1) * NSP]),
            tb_x_lo=np.ascontiguousarray(xt[:HALF]),
            tb_x_hi=np.ascontiguousarray(xt[HALF:]),
            hT0=hT0, xT0=xT0, nmaskT=nmT,
            **per_core[c], **warrs)
        in_maps.append(m)

    trace = os.environ.get("KTRACE", "0") == "1"
    res = run_bass_kernel_spmd(nc, in_maps, core_ids=list(range(NCORES)),
                               trace=trace)
    if trace:
        print("HW exec time:", res.exec_time_ns, "ns")
        if res.instructions_and_trace:
            print("trace path:", res.instructions_and_trace[1])
        kernel.last_result = res if False else None
    globals()["LAST_RES"] = res

    h_out = np.zeros((N, H), np.float32)
    x_out = np.zeros((N, 3), np.float32)
    for c in range(NCORES):
        r = res.results[c]
        h_out[c * NSR:(c + 1) * NSR] = r["hT_out"][:, :NSR].T
        x_out[c * NSR:(c + 1) * NSR] = r["xT_out"][:3, :NSR].T
    return (h_out, x_out)
